# revision 17
# baseline (speedup 1.0000x reference)
"""MoE (top-2 of 16 routed experts + 2 shared experts) Trainium2 kernel.

Strategy: expert-parallel over 8 NeuronCores, with token dispatch done on
host (the router is 0.01% of the FLOPs; computing it host-side lets each
core receive exactly the tokens it needs, already gathered, transposed and
packed for single-descriptor-per-partition DMA).

Per core (SPMD program, identical on all cores; per-core in_maps differ):
  slot "r0": routed expert A (its tokens, padded to R0_CAP)  -- fp8
  slot "r1": routed expert B (padded to R1_CAP)              -- fp8
  slot "sh": one shared expert on one quarter of tokens      -- bf16
Each slot is a dense FFN in feature-major ("transposed") layout:
  mid^T[d,t] = gelu( sum_h Wup[h,d] * x^T[h,t] )   (fp32 psum)
  y^T[h,t]   =       sum_d Wdn[d,h] * mid^T[d,t]
Host scatter-adds y^T into the [T,H] output with the router gate weights
(gelu is the only nonlinearity, so the per-token gate commutes with the
down projection).

The routed slots run in fp8 e4m3 with DoubleRow matmuls (2 contraction
rows per PE cell -> ~1.5x bf16 throughput).  The routed contribution is
damped by the top-2 gate weights (|routed| ~ 0.2 of |output|), so fp8
error there stays well inside the tolerance while the dominant shared
path stays bf16.  fp8 scaling: x*SX and W*SW on host (keeps values out
of the e4m3 subnormal range); the up-psum is descaled by the activation's
input scale, the down-projection's SW rides into y and is divided out of
the host-side gate weights.

All DRAM tensors are packed host-side as [128, free] with each partition's
data contiguous (x/y additionally chunk-major), so every DMA moves large
contiguous blocks per partition.
"""

import numpy as np
import ml_dtypes

import concourse.mybir as mybir
import concourse.tile as tile
from concourse import bacc
from concourse.bass_utils import run_bass_kernel_spmd

BF16 = mybir.dt.bfloat16
F8 = mybir.dt.float8e4
F32 = mybir.dt.float32
NP_BF16 = ml_dtypes.bfloat16
NP_F8 = ml_dtypes.float8_e4m3
DR = mybir.MatmulPerfMode.DoubleRow

B, S, H, D = 4, 1024, 2048, 1024
T = B * S
E_RT, E_SH, CORES = 16, 2, 8
HT, DT = H // 128, D // 128  # h-tiles (16), d-tiles (8)
N_SH = T // (CORES // E_SH)  # shared-slot tokens per core (1024)
TT = 512                     # max moving-operand / psum tile width
YG = 2                       # output h-tiles staged per store DMA
SX = 4.0                     # fp8 scale on routed x
SW = 64.0                    # fp8 scale on routed weights
WARM_MM = 26                 # PE warm-up matmuls

_prog_cache = {}
LAST_RESULTS = None  # BassKernelResults of the most recent run (for test.py)


def _chunks(n):
    """Split n into the fewest near-equal chunks of width <= TT."""
    k = -(-n // TT)
    base, rem = divmod(n, k)
    widths = [base + (1 if i < rem else 0) for i in range(k)]
    out, off = [], 0
    for w in widths:
        out.append((off, w))
        off += w
    return out


def _build_program(r0_cap, r1_cap):
    nc = bacc.Bacc("TRN2", target_bir_lowering=False, debug=False,
                   num_devices=CORES)
    slots = []
    for name, n, dt_in in (("sh", N_SH, BF16), ("r0", r0_cap, F8),
                           ("r1", r1_cap, F8)):
        xd = nc.dram_tensor(f"x_{name}", [128, HT * n], dt_in,
                            kind="ExternalInput")
        wu = nc.dram_tensor(f"wup_{name}", [128, HT * D], dt_in,
                            kind="ExternalInput")
        wd = nc.dram_tensor(f"wdn_{name}", [128, DT * H], dt_in,
                            kind="ExternalInput")
        yd = nc.dram_tensor(f"y_{name}", [128, HT * n], BF16,
                            kind="ExternalOutput")
        slots.append((name, n, dt_in, xd, wu, wd, yd))

    with tile.TileContext(nc) as tc:
        with (
            tc.tile_pool(name="wshpool", bufs=1) as wshpool,
            tc.tile_pool(name="wrtpool", bufs=2) as wrtpool,
            tc.tile_pool(name="xpool", bufs=3) as xpool,
            tc.tile_pool(name="mpool", bufs=2) as mpool,
            tc.tile_pool(name="mshpool", bufs=2) as mshpool,
            tc.tile_pool(name="ypool", bufs=3) as ypool,
            tc.tile_pool(name="ps1pool", bufs=4, space="PSUM") as ps1pool,
            tc.tile_pool(name="ps2pool", bufs=4, space="PSUM") as ps2pool,
        ):
            # DMA orchestration.  Two HWDGE rings exist (issued via SP and
            # ACT); each processes its DMAs in issue order, and an issue
            # blocks while its destination pool slot is busy.  The sync
            # ring carries loads only; the ACT ring carries slot-0's
            # other half plus the sh up-weights, then all y stores.
            def wtiles(si):
                name, n, dt_in, xd, wu, wd, yd = slots[si]
                pool = wshpool if name == "sh" else wrtpool
                tag = "sh" if name == "sh" else "rt"
                wut = pool.tile([128, HT, D], dt_in, tag=f"wup_{tag}",
                                name=f"wup_{name}")
                wdt = pool.tile([128, DT, H], dt_in, tag=f"wdn_{tag}",
                                name=f"wdn_{name}")
                return wut, wdt

            def load_x(si, ci, eng, halves=False):
                name, n, dt_in, xd, wu, wd, yd = slots[si]
                off, w = _chunks(n)[ci]
                xt = xpool.tile([128, HT, w], dt_in, tag="x",
                                name=f"x_{name}_{off}")
                hh = HT // 2
                pieces = ([(0, hh), (hh, HT - hh)] if halves
                          else [(0, HT)])
                for h0, hn in pieces:
                    eng.dma_start(
                        out=xt[:, h0:h0 + hn, :],
                        in_=xd[:, HT * off + h0 * w:
                               HT * off + (h0 + hn) * w].rearrange(
                            "p (h w) -> p h w", h=hn))
                return xt

            def load_wup(si, wut, gi, eng, hg=4):
                wu = slots[si][4]
                g = gi * hg
                eng.dma_start(
                    out=wut[:, g:g + hg, :],
                    in_=wu[:, g * D:(g + hg) * D].rearrange(
                        "p (h d) -> p h d", h=hg))

            def load_wdn(si, wdt, gi, eng, dg=4):
                wd = slots[si][5]
                g = gi * dg
                eng.dma_start(
                    out=wdt[:, g:g + dg, :],
                    in_=wd[:, g * H:(g + dg) * H].rearrange(
                        "p (c h) -> p c h", c=dg))

            # Dummy matmuls on scratch tiles: the PE HAM clock-gate only
            # lifts to 2.4 GHz after ~3.4us of sustained activity, so warm
            # it up while the first loads stream in.
            wlhs = xpool.tile([128, 128], BF16, tag="warm_l", bufs=1,
                              name="warm_lhs")
            wrhs = xpool.tile([128, TT], BF16, tag="warm_r", bufs=1,
                              name="warm_rhs")
            nc.vector.memset(wlhs[:], 0)
            nc.vector.memset(wrhs[:], 0)
            wps = ps1pool.tile([128, TT], F32, tag="ps1", name="warm_ps")
            for wi in range(WARM_MM):
                nc.tensor.matmul(wps[:], lhsT=wlhs[:], rhs=wrhs[:],
                                 start=True, stop=True)

            # ---- load emission (all upfront; pools give backpressure) ----
            w_sh, wd_sh = wtiles(0)
            w_r0, wd_r0 = wtiles(1)
            w_r1, wd_r1 = wtiles(2)
            # slot sh first: its 6MB critical set (up-weights + x chunk 0)
            # splits across both rings in consumption order; everything
            # else rides the sync ring ordered by deadline.  The ACT ring
            # stays clear of prefetch once compute begins.
            w0 = _chunks(N_SH)[0][1]
            xd_sh = slots[0][3]
            xt_sh0 = xpool.tile([128, HT, w0], BF16, tag="x",
                                name="x_sh_0")

            def xq(q, eng):
                eng.dma_start(
                    out=xt_sh0[:, 4 * q:4 * q + 4, :],
                    in_=xd_sh[:, 4 * q * w0:(4 * q + 4) * w0].rearrange(
                        "p (h w) -> p h w", h=4))

            # interleave x quarters with wup groups across both rings in
            # h-group consumption order (the first sh chunk runs h-group
            # outer, so compute starts after just xq0+g0 ~1.5MB)
            xq(0, nc.sync)
            load_wup(0, w_sh, 0, nc.scalar)
            load_wup(0, w_sh, 1, nc.sync)
            xq(1, nc.scalar)
            xq(2, nc.sync)
            load_wup(0, w_sh, 2, nc.scalar)
            load_wup(0, w_sh, 3, nc.sync)
            xq(3, nc.scalar)
            load_wdn(0, wd_sh, 0, nc.scalar)
            x_sh = [xt_sh0, load_x(0, 1, nc.sync)]
            load_wdn(0, wd_sh, 1, nc.sync)
            # routed slots: deadline-ordered on sync
            # routed-slot weights have large deadline slack, so load them
            # in the fewest DMAs -- each DMA's semaphore costs ~175ns in the
            # end-of-kernel release ceremony across all five engines.
            load_wup(1, w_r0, 0, nc.sync, hg=8)
            load_wup(1, w_r0, 1, nc.sync, hg=8)
            x_r0 = [load_x(1, ci, nc.sync)
                    for ci in range(len(_chunks(r0_cap)))]
            load_wdn(1, wd_r0, 0, nc.sync, dg=8)
            load_wup(2, w_r1, 0, nc.sync, hg=8)
            load_wup(2, w_r1, 1, nc.sync, hg=8)
            x_r1 = [load_x(2, ci, nc.sync)
                    for ci in range(len(_chunks(r1_cap)))]
            load_wdn(2, wd_r1, 0, nc.sync, dg=8)

            slot_tiles = [(w_sh, wd_sh, x_sh), (w_r0, wd_r0, x_r0),
                          (w_r1, wd_r1, x_r1)]

            def fp8_slot(si):
                """fp8 DoubleRow FFN; chunks interleaved so consecutive
                matmuls share the stationary weights."""
                name, n, dt_in, xd, wu, wd, yd = slots[si]
                wut, wdt, xts = slot_tiles[si]
                chs = _chunks(n)
                scale = 1.0 / (SX * SW)
                mids = []
                for ci, (off, w) in enumerate(chs):
                    mids.append(mpool.tile([128, DT, w], F8, tag="mid_rt",
                                           name=f"mid_{name}_{off}"))
                for dj in range(DT):
                    pss = [ps1pool.tile([128, TT], F32, tag="ps1",
                                        name=f"ps1_{name}_{ci}_{dj}")
                           for ci in range(len(chs))]
                    for hp in range(HT // 2):
                        for ci, (off, w) in enumerate(chs):
                            nc.tensor.matmul(
                                pss[ci][:, :w],
                                lhsT=wut[:, 2 * hp:2 * hp + 2,
                                         dj * 128:(dj + 1) * 128],
                                rhs=xts[ci][:, 2 * hp:2 * hp + 2, :],
                                start=(hp == 0),
                                stop=(hp == HT // 2 - 1),
                                perf_mode=DR,
                            )
                    for ci, (off, w) in enumerate(chs):
                        nc.scalar.activation(
                            mids[ci][:, dj, :], pss[ci][:, :w],
                            mybir.ActivationFunctionType.Gelu,
                            scale=scale)

                yts = [None] * len(chs)
                for hi in range(HT):
                    pss = [ps2pool.tile([128, TT], F32, tag="ps2",
                                        name=f"ps2_{name}_{ci}_{hi}")
                           for ci in range(len(chs))]
                    for dp in range(DT // 2):
                        for ci, (off, w) in enumerate(chs):
                            nc.tensor.matmul(
                                pss[ci][:, :w],
                                lhsT=wdt[:, 2 * dp:2 * dp + 2,
                                         hi * 128:(hi + 1) * 128],
                                rhs=mids[ci][:, 2 * dp:2 * dp + 2, :w],
                                start=(dp == 0),
                                stop=(dp == DT // 2 - 1),
                                perf_mode=DR,
                            )
                    g = hi % YG
                    for ci, (off, w) in enumerate(chs):
                        if g == 0:
                            yts[ci] = ypool.tile([128, YG, w], BF16,
                                                 tag="y",
                                                 name=f"y_{name}_{off}_{hi}")
                        nc.vector.tensor_copy(yts[ci][:, g, :],
                                              pss[ci][:, :w])
                        if g == YG - 1:
                            lo = HT * off + (hi - g) * w
                            nc.gpsimd.dma_start(
                                out=yd[:, lo:lo + YG * w].rearrange(
                                    "p (h w) -> p h w", h=YG),
                                in_=yts[ci][:])

            def bf16_slot(si):
                """bf16 FFN, chunk-sequential (pipelines chunk i down with
                chunk i+1 up)."""
                name, n, dt_in, xd, wu, wd, yd = slots[si]
                wut, wdt, xts = slot_tiles[si]
                for ci, (off, w) in enumerate(_chunks(n)):
                    base = HT * off
                    xt = xts[ci]
                    mid = mshpool.tile([128, DT, w], BF16, tag="mid_sh",
                                       name=f"mid_{name}_{off}")
                    if si == 0 and ci == 0:
                        # first chunk of the first slot: h-group outer with
                        # all DT psums live (across both psum pools), so the
                        # first matmul needs only the first x quarter + wup
                        # group instead of the full 6MB critical set.
                        pss = []
                        for dj in range(DT):
                            pool = ps1pool if dj < DT // 2 else ps2pool
                            tag = "ps1" if dj < DT // 2 else "ps2"
                            pss.append(pool.tile(
                                [128, TT], F32, tag=tag,
                                name=f"ps1_{name}_{off}_{dj}"))
                        for hg in range(4):
                            for dj in range(DT):
                                for hi in range(4 * hg, 4 * hg + 4):
                                    nc.tensor.matmul(
                                        pss[dj][:, :w],
                                        lhsT=wut[:, hi,
                                                 dj * 128:(dj + 1) * 128],
                                        rhs=xt[:, hi, :],
                                        start=(hi == 0),
                                        stop=(hi == HT - 1),
                                    )
                                if hg == 3:
                                    # drain each psum as soon as its last
                                    # h-group lands, so the activations
                                    # overlap the remaining matmuls instead
                                    # of bunching up and gating the down
                                    # phase
                                    nc.scalar.activation(
                                        mid[:, dj, :], pss[dj][:, :w],
                                        mybir.ActivationFunctionType.Gelu)
                    else:
                        for dj in range(DT):
                            ps = ps1pool.tile([128, TT], F32, tag="ps1",
                                              name=f"ps1_{name}_{off}_{dj}")
                            for hi in range(HT):
                                nc.tensor.matmul(
                                    ps[:, :w],
                                    lhsT=wut[:, hi, dj * 128:(dj + 1) * 128],
                                    rhs=xt[:, hi, :],
                                    start=(hi == 0),
                                    stop=(hi == HT - 1),
                                )
                            nc.scalar.activation(
                                mid[:, dj, :], ps[:, :w],
                                mybir.ActivationFunctionType.Gelu)

                    for hi in range(HT):
                        ps2 = ps2pool.tile([128, TT], F32, tag="ps2",
                                           name=f"ps2_{name}_{off}_{hi}")
                        for dj in range(DT):
                            nc.tensor.matmul(
                                ps2[:, :w],
                                lhsT=wdt[:, dj, hi * 128:(hi + 1) * 128],
                                rhs=mid[:, dj, :],
                                start=(dj == 0),
                                stop=(dj == DT - 1),
                            )
                        g = hi % YG
                        if g == 0:
                            yt = ypool.tile([128, YG, w], BF16, tag="y",
                                            name=f"y_{name}_{off}_{hi}")
                        nc.vector.tensor_copy(yt[:, g, :], ps2[:, :w])
                        if g == YG - 1:
                            lo = base + (hi - g) * w
                            nc.gpsimd.dma_start(
                                out=yd[:, lo:lo + YG * w].rearrange(
                                    "p (h w) -> p h w", h=YG),
                                in_=yt[:])

            bf16_slot(0)
            fp8_slot(1)
            fp8_slot(2)
    nc.compile()
    return nc


def _pack_rows(a, nt):
    """[nt*128, m] row-major -> [128, nt*m] with per-partition contiguous
    (tile-major) layout."""
    m = a.shape[1]
    return np.ascontiguousarray(
        a.reshape(nt, 128, m).transpose(1, 0, 2).reshape(128, nt * m))


def _pack_x(xTc):
    """[H, n] -> [128, HT*n] chunk-major."""
    n = xTc.shape[1]
    parts = [_pack_rows(xTc[:, off:off + w], HT) for off, w in _chunks(n)]
    return np.ascontiguousarray(np.concatenate(parts, axis=1))


def _unpack_y(yflat, n):
    """[128, HT*n] chunk-major -> [n, H] (token-major)."""
    yflat = yflat.astype(np.float32)
    out = np.empty((n, H), np.float32)
    base = 0
    for off, w in _chunks(n):
        blk = yflat[:, base:base + HT * w].reshape(128, HT, w)
        out[off:off + w] = blk.transpose(2, 1, 0).reshape(w, H)
        base += HT * w
    return out


def _route(x2d, w_router):
    """Top-2 routing, matching the reference's softmax-then-top_k."""
    logits = x2d @ w_router
    m = logits.max(-1, keepdims=True)
    e = np.exp(logits - m)
    probs = e / e.sum(-1, keepdims=True)
    rows = np.arange(x2d.shape[0])
    i1 = probs.argmax(-1)
    masked = probs.copy()
    masked[rows, i1] = -np.inf
    i2 = masked.argmax(-1)
    return probs, i1, i2


def kernel(x, Wsh_up, Wsh_down, Wrt_up, Wrt_down, W_router):
    global LAST_RESULTS
    x = np.asarray(x, np.float32)
    Wsh_up = np.asarray(Wsh_up, np.float32)
    Wsh_down = np.asarray(Wsh_down, np.float32)
    Wrt_up = np.asarray(Wrt_up, np.float32)
    Wrt_down = np.asarray(Wrt_down, np.float32)
    W_router = np.asarray(W_router, np.float32)

    x2d = x.reshape(T, H)
    probs, i1, i2 = _route(x2d, W_router)

    # token ids / gate values per routed expert
    ids, gates = [], []
    for e in range(E_RT):
        sel = np.where((i1 == e) | (i2 == e))[0]
        ids.append(sel)
        gates.append(probs[sel, e].astype(np.float32) / SW)

    # slot r0 takes the 8 most-loaded experts, r1 the 8 least-loaded, so
    # the two static capacities hug the actual counts.
    order = sorted(range(E_RT), key=lambda e: -len(ids[e]))
    slot_experts = {0: order[:CORES], 1: order[CORES:]}
    caps = []
    for slot in range(2):
        mx = max(len(ids[e]) for e in slot_experts[slot])
        caps.append(max(512, -(-mx // 32) * 32))
    r0_cap, r1_cap = caps

    key = (r0_cap, r1_cap)
    if key not in _prog_cache:
        _prog_cache[key] = _build_program(r0_cap, r1_cap)
    nc = _prog_cache[key]

    xbf = x2d.astype(NP_BF16)
    xq = (x2d * SX).astype(NP_F8)
    wup_sh = Wsh_up.astype(NP_BF16)
    wdn_sh = Wsh_down.astype(NP_BF16)
    wup_rt = (Wrt_up * SW).astype(NP_F8)
    wdn_rt = (Wrt_down * SW).astype(NP_F8)

    in_maps = []
    for c in range(CORES):
        se, q = c % E_SH, c // E_SH
        m = {
            "x_sh": _pack_x(np.ascontiguousarray(
                xbf[q * N_SH:(q + 1) * N_SH].T)),
            "wup_sh": _pack_rows(wup_sh[se], HT),
            "wdn_sh": _pack_rows(wdn_sh[se], DT),
        }
        for slot, cap in ((0, r0_cap), (1, r1_cap)):
            e = slot_experts[slot][c]
            sel = ids[e]
            xe = np.zeros((H, cap), NP_F8)
            xe[:, :len(sel)] = xq[sel].T
            m[f"x_r{slot}"] = _pack_x(xe)
            m[f"wup_r{slot}"] = _pack_rows(wup_rt[e], HT)
            m[f"wdn_r{slot}"] = _pack_rows(wdn_rt[e], DT)
        in_maps.append(m)

    res = None
    for attempt in range(3):
        try:
            res = run_bass_kernel_spmd(nc, in_maps,
                                       core_ids=list(range(CORES)))
            break
        except Exception:
            if attempt == 2:
                raise
    LAST_RESULTS = res

    out = np.zeros((T, H), np.float32)
    for c in range(CORES):
        q = c // E_SH
        out[q * N_SH:(q + 1) * N_SH] += _unpack_y(res.results[c]["y_sh"],
                                                  N_SH)
    for slot, cap in ((0, r0_cap), (1, r1_cap)):
        for c in range(CORES):
            e = slot_experts[slot][c]
            sel = ids[e]
            y = _unpack_y(res.results[c][f"y_r{slot}"], cap)
            out[sel] += gates[e][:, None] * y[:len(sel)]
    return out.reshape(B, S, H)


# revision 19
# speedup vs baseline: 1.0043x; 1.0043x over previous
"""MoE (top-2 of 16 routed experts + 2 shared experts) Trainium2 kernel.

Strategy: expert-parallel over 8 NeuronCores, with token dispatch done on
host (the router is 0.01% of the FLOPs; computing it host-side lets each
core receive exactly the tokens it needs, already gathered, transposed and
packed for single-descriptor-per-partition DMA).

Per core (SPMD program, identical on all cores; per-core in_maps differ):
  slot "r0": routed expert A (its tokens, padded to R0_CAP)  -- fp8
  slot "r1": routed expert B (padded to R1_CAP)              -- fp8
  slot "sh": one shared expert on one quarter of tokens      -- bf16
Each slot is a dense FFN in feature-major ("transposed") layout:
  mid^T[d,t] = gelu( sum_h Wup[h,d] * x^T[h,t] )   (fp32 psum)
  y^T[h,t]   =       sum_d Wdn[d,h] * mid^T[d,t]
Host scatter-adds y^T into the [T,H] output with the router gate weights
(gelu is the only nonlinearity, so the per-token gate commutes with the
down projection).

The routed slots run in fp8 e4m3 with DoubleRow matmuls (2 contraction
rows per PE cell -> ~1.5x bf16 throughput).  The routed contribution is
damped by the top-2 gate weights (|routed| ~ 0.2 of |output|), so fp8
error there stays well inside the tolerance while the dominant shared
path stays bf16.  fp8 scaling: x*SX and W*SW on host (keeps values out
of the e4m3 subnormal range); the up-psum is descaled by the activation's
input scale, the down-projection's SW rides into y and is divided out of
the host-side gate weights.

All DRAM tensors are packed host-side as [128, free] with each partition's
data contiguous (x/y additionally chunk-major), so every DMA moves large
contiguous blocks per partition.
"""

import numpy as np
import ml_dtypes

import concourse.mybir as mybir
import concourse.tile as tile
from concourse import bacc
from concourse.bass_utils import run_bass_kernel_spmd

BF16 = mybir.dt.bfloat16
F8 = mybir.dt.float8e4
F32 = mybir.dt.float32
NP_BF16 = ml_dtypes.bfloat16
NP_F8 = ml_dtypes.float8_e4m3
DR = mybir.MatmulPerfMode.DoubleRow

B, S, H, D = 4, 1024, 2048, 1024
T = B * S
E_RT, E_SH, CORES = 16, 2, 8
HT, DT = H // 128, D // 128  # h-tiles (16), d-tiles (8)
N_SH = T // (CORES // E_SH)  # shared-slot tokens per core (1024)
TT = 512                     # max moving-operand / psum tile width
YG = 2                       # output h-tiles staged per store DMA
SX = 4.0                     # fp8 scale on routed x
SW = 64.0                    # fp8 scale on routed weights
WARM_MM = 8                 # PE warm-up matmuls

_prog_cache = {}
LAST_RESULTS = None  # BassKernelResults of the most recent run (for test.py)


def _chunks(n):
    """Split n into the fewest near-equal chunks of width <= TT."""
    k = -(-n // TT)
    base, rem = divmod(n, k)
    widths = [base + (1 if i < rem else 0) for i in range(k)]
    out, off = [], 0
    for w in widths:
        out.append((off, w))
        off += w
    return out


def _build_program(r0_cap, r1_cap):
    nc = bacc.Bacc("TRN2", target_bir_lowering=False, debug=False,
                   num_devices=CORES)
    slots = []
    for name, n, dt_in in (("sh", N_SH, BF16), ("r0", r0_cap, F8),
                           ("r1", r1_cap, F8)):
        xd = nc.dram_tensor(f"x_{name}", [128, HT * n], dt_in,
                            kind="ExternalInput")
        wu = nc.dram_tensor(f"wup_{name}", [128, HT * D], dt_in,
                            kind="ExternalInput")
        wd = nc.dram_tensor(f"wdn_{name}", [128, DT * H], dt_in,
                            kind="ExternalInput")
        yd = nc.dram_tensor(f"y_{name}", [128, HT * n], BF16,
                            kind="ExternalOutput")
        slots.append((name, n, dt_in, xd, wu, wd, yd))

    with tile.TileContext(nc) as tc:
        with (
            tc.tile_pool(name="wshpool", bufs=1) as wshpool,
            tc.tile_pool(name="wrtpool", bufs=2) as wrtpool,
            tc.tile_pool(name="xpool", bufs=3) as xpool,
            tc.tile_pool(name="mpool", bufs=2) as mpool,
            tc.tile_pool(name="mshpool", bufs=2) as mshpool,
            tc.tile_pool(name="ypool", bufs=3) as ypool,
            tc.tile_pool(name="ps1pool", bufs=4, space="PSUM") as ps1pool,
            tc.tile_pool(name="ps2pool", bufs=4, space="PSUM") as ps2pool,
        ):
            # DMA orchestration.  Two HWDGE rings exist (issued via SP and
            # ACT); each processes its DMAs in issue order, and an issue
            # blocks while its destination pool slot is busy.  The sync
            # ring carries loads only; the ACT ring carries slot-0's
            # other half plus the sh up-weights, then all y stores.
            def wtiles(si):
                name, n, dt_in, xd, wu, wd, yd = slots[si]
                pool = wshpool if name == "sh" else wrtpool
                tag = "sh" if name == "sh" else "rt"
                wut = pool.tile([128, HT, D], dt_in, tag=f"wup_{tag}",
                                name=f"wup_{name}")
                wdt = pool.tile([128, DT, H], dt_in, tag=f"wdn_{tag}",
                                name=f"wdn_{name}")
                return wut, wdt

            def load_x(si, ci, eng, halves=False):
                name, n, dt_in, xd, wu, wd, yd = slots[si]
                off, w = _chunks(n)[ci]
                xt = xpool.tile([128, HT, w], dt_in, tag="x",
                                name=f"x_{name}_{off}")
                hh = HT // 2
                pieces = ([(0, hh), (hh, HT - hh)] if halves
                          else [(0, HT)])
                for h0, hn in pieces:
                    eng.dma_start(
                        out=xt[:, h0:h0 + hn, :],
                        in_=xd[:, HT * off + h0 * w:
                               HT * off + (h0 + hn) * w].rearrange(
                            "p (h w) -> p h w", h=hn))
                return xt

            def load_wup(si, wut, gi, eng, hg=4):
                wu = slots[si][4]
                g = gi * hg
                eng.dma_start(
                    out=wut[:, g:g + hg, :],
                    in_=wu[:, g * D:(g + hg) * D].rearrange(
                        "p (h d) -> p h d", h=hg))

            def load_wdn(si, wdt, gi, eng):
                wd = slots[si][5]
                dg = 4
                g = gi * dg
                eng.dma_start(
                    out=wdt[:, g:g + dg, :],
                    in_=wd[:, g * H:(g + dg) * H].rearrange(
                        "p (c h) -> p c h", c=dg))

            # Dummy matmuls on scratch tiles: the PE HAM clock-gate only
            # lifts to 2.4 GHz after ~3.4us of sustained activity, so warm
            # it up while the first loads stream in.
            wlhs = xpool.tile([128, 128], BF16, tag="warm_l", bufs=1,
                              name="warm_lhs")
            wrhs = xpool.tile([128, TT], BF16, tag="warm_r", bufs=1,
                              name="warm_rhs")
            nc.vector.memset(wlhs[:], 0)
            nc.vector.memset(wrhs[:], 0)
            wps = ps1pool.tile([128, TT], F32, tag="ps1", name="warm_ps")
            for wi in range(WARM_MM):
                nc.tensor.matmul(wps[:], lhsT=wlhs[:], rhs=wrhs[:],
                                 start=True, stop=True)

            # ---- load emission (all upfront; pools give backpressure) ----
            w_sh, wd_sh = wtiles(0)
            w_r0, wd_r0 = wtiles(1)
            w_r1, wd_r1 = wtiles(2)
            # slot sh first: its 6MB critical set (up-weights + x chunk 0)
            # splits across both rings in consumption order; everything
            # else rides the sync ring ordered by deadline.  The ACT ring
            # stays clear of prefetch once compute begins.
            w0 = _chunks(N_SH)[0][1]
            xd_sh = slots[0][3]
            xt_sh0 = xpool.tile([128, HT, w0], BF16, tag="x",
                                name="x_sh_0")

            def xq(q, eng):
                eng.dma_start(
                    out=xt_sh0[:, 4 * q:4 * q + 4, :],
                    in_=xd_sh[:, 4 * q * w0:(4 * q + 4) * w0].rearrange(
                        "p (h w) -> p h w", h=4))

            # interleave x quarters with wup groups across both rings in
            # h-group consumption order (the first sh chunk runs h-group
            # outer, so compute starts after just xq0+g0 ~1.5MB)
            # the leading x quarter and wup group are split once more so
            # the very first matmuls (hi 0-1) can issue off the first half
            # while the second streams; real compute then starts at ~9us
            # (ramp clock) instead of waiting for full warmup coverage
            nc.sync.dma_start(
                out=xt_sh0[:, 0:2, :],
                in_=xd_sh[:, 0:2 * w0].rearrange("p (h w) -> p h w", h=2))
            eng_g0 = nc.scalar
            eng_g0.dma_start(
                out=w_sh[:, 0:2, :],
                in_=slots[0][4][:, 0:2 * D].rearrange(
                    "p (h d) -> p h d", h=2))
            nc.sync.dma_start(
                out=xt_sh0[:, 2:4, :],
                in_=xd_sh[:, 2 * w0:4 * w0].rearrange(
                    "p (h w) -> p h w", h=2))
            eng_g0.dma_start(
                out=w_sh[:, 2:4, :],
                in_=slots[0][4][:, 2 * D:4 * D].rearrange(
                    "p (h d) -> p h d", h=2))
            load_wup(0, w_sh, 1, nc.sync)
            xq(1, nc.scalar)
            xq(2, nc.sync)
            load_wup(0, w_sh, 2, nc.scalar)
            load_wup(0, w_sh, 3, nc.sync)
            xq(3, nc.scalar)
            load_wdn(0, wd_sh, 0, nc.scalar)
            x_sh = [xt_sh0, load_x(0, 1, nc.sync)]
            load_wdn(0, wd_sh, 1, nc.sync)
            # routed slots: deadline-ordered on sync
            for gi in range(4):
                load_wup(1, w_r0, gi, nc.sync)
            x_r0 = [load_x(1, ci, nc.sync)
                    for ci in range(len(_chunks(r0_cap)))]
            load_wdn(1, wd_r0, 0, nc.sync)
            load_wdn(1, wd_r0, 1, nc.sync)
            for gi in range(4):
                load_wup(2, w_r1, gi, nc.sync)
            x_r1 = [load_x(2, ci, nc.sync)
                    for ci in range(len(_chunks(r1_cap)))]
            load_wdn(2, wd_r1, 0, nc.sync)
            load_wdn(2, wd_r1, 1, nc.sync)

            slot_tiles = [(w_sh, wd_sh, x_sh), (w_r0, wd_r0, x_r0),
                          (w_r1, wd_r1, x_r1)]

            def fp8_slot(si):
                """fp8 DoubleRow FFN; chunks interleaved so consecutive
                matmuls share the stationary weights."""
                name, n, dt_in, xd, wu, wd, yd = slots[si]
                wut, wdt, xts = slot_tiles[si]
                chs = _chunks(n)
                scale = 1.0 / (SX * SW)
                mids = []
                for ci, (off, w) in enumerate(chs):
                    mids.append(mpool.tile([128, DT, w], F8, tag="mid_rt",
                                           name=f"mid_{name}_{off}"))
                for dj in range(DT):
                    pss = [ps1pool.tile([128, TT], F32, tag="ps1",
                                        name=f"ps1_{name}_{ci}_{dj}")
                           for ci in range(len(chs))]
                    for hp in range(HT // 2):
                        for ci, (off, w) in enumerate(chs):
                            nc.tensor.matmul(
                                pss[ci][:, :w],
                                lhsT=wut[:, 2 * hp:2 * hp + 2,
                                         dj * 128:(dj + 1) * 128],
                                rhs=xts[ci][:, 2 * hp:2 * hp + 2, :],
                                start=(hp == 0),
                                stop=(hp == HT // 2 - 1),
                                perf_mode=DR,
                            )
                    for ci, (off, w) in enumerate(chs):
                        nc.scalar.activation(
                            mids[ci][:, dj, :], pss[ci][:, :w],
                            mybir.ActivationFunctionType.Gelu,
                            scale=scale)

                yts = [None] * len(chs)
                for hi in range(HT):
                    pss = [ps2pool.tile([128, TT], F32, tag="ps2",
                                        name=f"ps2_{name}_{ci}_{hi}")
                           for ci in range(len(chs))]
                    for dp in range(DT // 2):
                        for ci, (off, w) in enumerate(chs):
                            nc.tensor.matmul(
                                pss[ci][:, :w],
                                lhsT=wdt[:, 2 * dp:2 * dp + 2,
                                         hi * 128:(hi + 1) * 128],
                                rhs=mids[ci][:, 2 * dp:2 * dp + 2, :w],
                                start=(dp == 0),
                                stop=(dp == DT // 2 - 1),
                                perf_mode=DR,
                            )
                    g = hi % YG
                    for ci, (off, w) in enumerate(chs):
                        if g == 0:
                            yts[ci] = ypool.tile([128, YG, w], BF16,
                                                 tag="y",
                                                 name=f"y_{name}_{off}_{hi}")
                        nc.vector.tensor_copy(yts[ci][:, g, :],
                                              pss[ci][:, :w])
                        if g == YG - 1:
                            lo = HT * off + (hi - g) * w
                            nc.gpsimd.dma_start(
                                out=yd[:, lo:lo + YG * w].rearrange(
                                    "p (h w) -> p h w", h=YG),
                                in_=yts[ci][:])

            def bf16_slot(si):
                """bf16 FFN, chunk-sequential (pipelines chunk i down with
                chunk i+1 up)."""
                name, n, dt_in, xd, wu, wd, yd = slots[si]
                wut, wdt, xts = slot_tiles[si]
                for ci, (off, w) in enumerate(_chunks(n)):
                    base = HT * off
                    xt = xts[ci]
                    mid = mshpool.tile([128, DT, w], BF16, tag="mid_sh",
                                       name=f"mid_{name}_{off}")
                    if si == 0 and ci == 0:
                        # first chunk of the first slot: h-group outer with
                        # all DT psums live (across both psum pools), so the
                        # first matmul needs only the first x quarter + wup
                        # group instead of the full 6MB critical set.
                        pss = []
                        for dj in range(DT):
                            pool = ps1pool if dj < DT // 2 else ps2pool
                            tag = "ps1" if dj < DT // 2 else "ps2"
                            pss.append(pool.tile(
                                [128, TT], F32, tag=tag,
                                name=f"ps1_{name}_{off}_{dj}"))
                        for hg in range(4):
                            for dj in range(DT):
                                for hi in range(4 * hg, 4 * hg + 4):
                                    nc.tensor.matmul(
                                        pss[dj][:, :w],
                                        lhsT=wut[:, hi,
                                                 dj * 128:(dj + 1) * 128],
                                        rhs=xt[:, hi, :],
                                        start=(hi == 0),
                                        stop=(hi == HT - 1),
                                    )
                                if hg == 3:
                                    # drain each psum as soon as its last
                                    # h-group lands, so the activations
                                    # overlap the remaining matmuls instead
                                    # of bunching up and gating the down
                                    # phase
                                    nc.scalar.activation(
                                        mid[:, dj, :], pss[dj][:, :w],
                                        mybir.ActivationFunctionType.Gelu)
                    else:
                        for dj in range(DT):
                            ps = ps1pool.tile([128, TT], F32, tag="ps1",
                                              name=f"ps1_{name}_{off}_{dj}")
                            for hi in range(HT):
                                nc.tensor.matmul(
                                    ps[:, :w],
                                    lhsT=wut[:, hi, dj * 128:(dj + 1) * 128],
                                    rhs=xt[:, hi, :],
                                    start=(hi == 0),
                                    stop=(hi == HT - 1),
                                )
                            nc.scalar.activation(
                                mid[:, dj, :], ps[:, :w],
                                mybir.ActivationFunctionType.Gelu)

                    for hi in range(HT):
                        ps2 = ps2pool.tile([128, TT], F32, tag="ps2",
                                           name=f"ps2_{name}_{off}_{hi}")
                        for dj in range(DT):
                            nc.tensor.matmul(
                                ps2[:, :w],
                                lhsT=wdt[:, dj, hi * 128:(hi + 1) * 128],
                                rhs=mid[:, dj, :],
                                start=(dj == 0),
                                stop=(dj == DT - 1),
                            )
                        g = hi % YG
                        if g == 0:
                            yt = ypool.tile([128, YG, w], BF16, tag="y",
                                            name=f"y_{name}_{off}_{hi}")
                        nc.vector.tensor_copy(yt[:, g, :], ps2[:, :w])
                        if g == YG - 1:
                            lo = base + (hi - g) * w
                            nc.gpsimd.dma_start(
                                out=yd[:, lo:lo + YG * w].rearrange(
                                    "p (h w) -> p h w", h=YG),
                                in_=yt[:])

            bf16_slot(0)
            fp8_slot(1)
            fp8_slot(2)
    nc.compile()
    return nc


def _pack_rows(a, nt):
    """[nt*128, m] row-major -> [128, nt*m] with per-partition contiguous
    (tile-major) layout."""
    m = a.shape[1]
    return np.ascontiguousarray(
        a.reshape(nt, 128, m).transpose(1, 0, 2).reshape(128, nt * m))


def _pack_x(xTc):
    """[H, n] -> [128, HT*n] chunk-major."""
    n = xTc.shape[1]
    parts = [_pack_rows(xTc[:, off:off + w], HT) for off, w in _chunks(n)]
    return np.ascontiguousarray(np.concatenate(parts, axis=1))


def _unpack_y(yflat, n):
    """[128, HT*n] chunk-major -> [n, H] (token-major)."""
    yflat = yflat.astype(np.float32)
    out = np.empty((n, H), np.float32)
    base = 0
    for off, w in _chunks(n):
        blk = yflat[:, base:base + HT * w].reshape(128, HT, w)
        out[off:off + w] = blk.transpose(2, 1, 0).reshape(w, H)
        base += HT * w
    return out


def _route(x2d, w_router):
    """Top-2 routing, matching the reference's softmax-then-top_k."""
    logits = x2d @ w_router
    m = logits.max(-1, keepdims=True)
    e = np.exp(logits - m)
    probs = e / e.sum(-1, keepdims=True)
    rows = np.arange(x2d.shape[0])
    i1 = probs.argmax(-1)
    masked = probs.copy()
    masked[rows, i1] = -np.inf
    i2 = masked.argmax(-1)
    return probs, i1, i2


def kernel(x, Wsh_up, Wsh_down, Wrt_up, Wrt_down, W_router):
    global LAST_RESULTS
    x = np.asarray(x, np.float32)
    Wsh_up = np.asarray(Wsh_up, np.float32)
    Wsh_down = np.asarray(Wsh_down, np.float32)
    Wrt_up = np.asarray(Wrt_up, np.float32)
    Wrt_down = np.asarray(Wrt_down, np.float32)
    W_router = np.asarray(W_router, np.float32)

    x2d = x.reshape(T, H)
    probs, i1, i2 = _route(x2d, W_router)

    # token ids / gate values per routed expert
    ids, gates = [], []
    for e in range(E_RT):
        sel = np.where((i1 == e) | (i2 == e))[0]
        ids.append(sel)
        gates.append(probs[sel, e].astype(np.float32) / SW)

    # slot r0 takes the 8 most-loaded experts, r1 the 8 least-loaded, so
    # the two static capacities hug the actual counts.
    order = sorted(range(E_RT), key=lambda e: -len(ids[e]))
    slot_experts = {0: order[:CORES], 1: order[CORES:]}
    caps = []
    for slot in range(2):
        mx = max(len(ids[e]) for e in slot_experts[slot])
        caps.append(max(512, -(-mx // 32) * 32))
    r0_cap, r1_cap = caps

    key = (r0_cap, r1_cap)
    if key not in _prog_cache:
        _prog_cache[key] = _build_program(r0_cap, r1_cap)
    nc = _prog_cache[key]

    xbf = x2d.astype(NP_BF16)
    xq = (x2d * SX).astype(NP_F8)
    wup_sh = Wsh_up.astype(NP_BF16)
    wdn_sh = Wsh_down.astype(NP_BF16)
    wup_rt = (Wrt_up * SW).astype(NP_F8)
    wdn_rt = (Wrt_down * SW).astype(NP_F8)

    in_maps = []
    for c in range(CORES):
        se, q = c % E_SH, c // E_SH
        m = {
            "x_sh": _pack_x(np.ascontiguousarray(
                xbf[q * N_SH:(q + 1) * N_SH].T)),
            "wup_sh": _pack_rows(wup_sh[se], HT),
            "wdn_sh": _pack_rows(wdn_sh[se], DT),
        }
        for slot, cap in ((0, r0_cap), (1, r1_cap)):
            e = slot_experts[slot][c]
            sel = ids[e]
            xe = np.zeros((H, cap), NP_F8)
            xe[:, :len(sel)] = xq[sel].T
            m[f"x_r{slot}"] = _pack_x(xe)
            m[f"wup_r{slot}"] = _pack_rows(wup_rt[e], HT)
            m[f"wdn_r{slot}"] = _pack_rows(wdn_rt[e], DT)
        in_maps.append(m)

    res = None
    for attempt in range(3):
        try:
            res = run_bass_kernel_spmd(nc, in_maps,
                                       core_ids=list(range(CORES)))
            break
        except Exception:
            if attempt == 2:
                raise
    LAST_RESULTS = res

    out = np.zeros((T, H), np.float32)
    for c in range(CORES):
        q = c // E_SH
        out[q * N_SH:(q + 1) * N_SH] += _unpack_y(res.results[c]["y_sh"],
                                                  N_SH)
    for slot, cap in ((0, r0_cap), (1, r1_cap)):
        for c in range(CORES):
            e = slot_experts[slot][c]
            sel = ids[e]
            y = _unpack_y(res.results[c][f"y_r{slot}"], cap)
            out[sel] += gates[e][:, None] * y[:len(sel)]
    return out.reshape(B, S, H)
